# revision 21
# baseline (speedup 1.0000x reference)
"""LoRA row-parallel linear on 8 TRN2 NeuronCores.

Problem: y = x @ W^T + delta, where per-token LoRA delta[t] = B[s] @ (A[s] @ x[t]),
s = token_to_slot[t] (8 adapters, rank 16, scaling baked into B).

Strategy: token data-parallel across the 8 cores (T=8192 -> 1024 tokens/core).
No collectives needed; each core computes its token block fully, in transposed
output space (y^T, un-transposed on the host):
  u^T   = A_all @ x_shard^T          (128 x T_SH; A_all = all 8 adapters stacked)
  uM^T  = u^T * mask^T               (one-hot select of each token's adapter)
  y^T   = W @ x^T + B_all^T @ uM^T   (PSUM accumulation: 32 k-tiles of W + 1 of B)
All matmuls run as float32r (TF32-like, FP22) at full PE rate with fp32
accumulate (measured ~227 ns per 128x128x512 matmul, the intrinsic pacing;
f32r matmuls are self-loading, so stationary-operand choice is neutral).

Schedule (per core): the first output-column block (ob0) runs its 32-k-tile
d-loop FIRST, so the PE has work while the 16 MB x^T shard streams in; the
u-pass (which needs the whole shard) runs after it, and ob0's LoRA delta is
applied as a separate accumulation + DVE add. Remaining obs fuse the delta as
a 33rd accumulation step.

Host prep: transposes x/W/A to put the contraction dim on partitions, builds
the one-hot mask from token_to_slot. Device does all the FLOPs.
"""

import numpy as np
import ml_dtypes

from concourse import bacc, tile, mybir
from concourse.bass_utils import run_bass_kernel_spmd
import concourse.bass_utils as _bu

# Disable S3 artifact upload in the trace path (no credentials in this container).
_bu.upload_artifacts = lambda tmpdir: "local://" + tmpdir

N_CORES = 8
T = 8192
D_IN = 4096
D_OUT = 4096
L = 8          # max adapters
R = 16         # max rank
LR = L * R     # 128 = stacked adapter dim
T_SH = T // N_CORES          # 1024 tokens per core
KT = D_IN // 128             # 32 contraction tiles
OB = D_OUT // 512            # 8 output-column superblocks
NO = 4                       # 128-wide output blocks per superblock
NT = T_SH // 512             # 2 token blocks (moving dim)

F32 = mybir.dt.float32
F32R = mybir.dt.float32r

_CACHED_NC = None


def _build():
    nc = bacc.Bacc("TRN2", target_bir_lowering=False, debug=False)

    xT_d = nc.dram_tensor("xT", [D_IN, T_SH], F32, kind="ExternalInput")
    wT_d = nc.dram_tensor("wT", [D_IN, D_OUT], F32, kind="ExternalInput")
    aT_d = nc.dram_tensor("aT", [D_IN, LR], F32, kind="ExternalInput")
    bC_d = nc.dram_tensor("bC", [LR, D_OUT], F32, kind="ExternalInput")
    mT_d = nc.dram_tensor("maskT", [LR, T_SH], mybir.dt.bfloat16, kind="ExternalInput")
    yT_d = nc.dram_tensor("yT", [D_OUT, T_SH], F32, kind="ExternalOutput")

    with tile.TileContext(nc) as tc:
        with (
            tc.tile_pool(name="resident", bufs=1) as rpool,
            tc.tile_pool(name="wstream", bufs=9) as wpool,
            tc.tile_pool(name="yout", bufs=3) as ypool,
            tc.tile_pool(name="psum", bufs=8, space="PSUM") as psum,
        ):
            # --- resident loads; xts interleaved with ob0's w tiles so the
            # --- ob0 d-loop can start as soon as the first k-tile lands.
            xts = []
            wts0 = []
            ats = []
            for d in range(KT):
                xt = rpool.tile([128, T_SH], F32R, tag=f"xt{d}")
                nc.sync.dma_start(xt[:], xT_d[d * 128:(d + 1) * 128, :].bitcast(F32R))
                xts.append(xt)
                wt = wpool.tile([128, 512], F32R, tag="wt", name=f"wt0_{d}")
                nc.sync.dma_start(wt[:], wT_d[d * 128:(d + 1) * 128, 0:512].bitcast(F32R))
                wts0.append(wt)
            for d in range(KT):
                at = rpool.tile([128, LR], F32R, tag=f"at{d}", name=f"at{d}")
                nc.sync.dma_start(at[:], aT_d[d * 128:(d + 1) * 128, :].bitcast(F32R))
                ats.append(at)
            bc = rpool.tile([LR, D_OUT], F32R, tag="bc")
            nc.sync.dma_start(bc[:], bC_d[:].bitcast(F32R))
            mask = rpool.tile([LR, T_SH], mybir.dt.bfloat16, tag="mask")
            nc.sync.dma_start(mask[:], mT_d[:])
            uTms = [rpool.tile([LR, 512], F32R, tag=f"uTm{ub}", name=f"uTm{ub}")
                    for ub in range(NT)]

            # --- phase 1: ob0 d-loop (base matmul only, no delta) --------------
            # psum tile (o, t) = y^T[o-block of 128, t-block of 512]
            pys0 = [[psum.tile([128, 512], F32, tag="acc", name=f"py0_{o}_{t}")
                     for t in range(NT)] for o in range(NO)]
            yo0s = {}
            for d in range(KT):
                for o in range(NO):
                    lw = wts0[d][:, o * 128:(o + 1) * 128]
                    for t in range(NT):
                        nc.tensor.matmul(
                            pys0[o][t][:], lw, xts[d][:, t * 512:(t + 1) * 512],
                            start=(d == 0), stop=(d == KT - 1), skip_group_check=True,
                        )
                        if d == KT - 1:
                            yo0 = rpool.tile([128, 512], F32, tag=f"yo0_{o}_{t}",
                                             name=f"yo0_{o}_{t}")
                            nc.vector.tensor_copy(yo0[:], pys0[o][t][:])
                            yo0s[o, t] = yo0

            # --- phase 2: u-pass (needs all xts, which have landed by now) -----
            for ub in range(NT):
                pu = psum.tile([128, 512], F32, tag="acc", name=f"pu{ub}")
                sl = slice(ub * 512, (ub + 1) * 512)
                for d in range(KT):
                    nc.tensor.matmul(
                        pu[:], ats[d][:], xts[d][:, sl],
                        start=(d == 0), stop=(d == KT - 1), skip_group_check=True,
                    )
                nc.vector.tensor_mul(uTms[ub][:], pu[:], mask[:, sl])

            # --- phase 3: ob0 delta + writeback --------------------------------
            for o in range(NO):
                for t in range(NT):
                    pd = psum.tile([128, 512], F32, tag="acc", name=f"pd{o}_{t}")
                    nc.tensor.matmul(
                        pd[:], bc[:, o * 128:(o + 1) * 128], uTms[t][:],
                        start=True, stop=True, skip_group_check=True,
                    )
                    yo = ypool.tile([128, 512], F32, tag="yo", name=f"yod{o}_{t}")
                    nc.vector.tensor_add(yo[:], yo0s[o, t][:], pd[:])
                    nc.sync.dma_start(
                        yT_d[o * 128:(o + 1) * 128, t * 512:(t + 1) * 512], yo[:])

            # --- phase 4: ob1..7 with fused delta ------------------------------
            for ob in range(1, OB):
                pys = [[psum.tile([128, 512], F32, tag="acc", name=f"py{ob}_{o}_{t}")
                        for t in range(NT)] for o in range(NO)]
                for d in range(KT):
                    wt = wpool.tile([128, 512], F32R, tag="wt", name=f"wt{ob}_{d}")
                    nc.sync.dma_start(
                        wt[:],
                        wT_d[d * 128:(d + 1) * 128,
                             ob * 512:(ob + 1) * 512].bitcast(F32R))
                    for o in range(NO):
                        lw = wt[:, o * 128:(o + 1) * 128]
                        og = ob * 512 + o * 128
                        for t in range(NT):
                            nc.tensor.matmul(
                                pys[o][t][:], lw, xts[d][:, t * 512:(t + 1) * 512],
                                start=(d == 0), stop=False, skip_group_check=True,
                            )
                            if d == KT - 1:
                                nc.tensor.matmul(
                                    pys[o][t][:], bc[:, og:og + 128], uTms[t][:],
                                    start=False, stop=True, skip_group_check=True,
                                )
                                yo = ypool.tile([128, 512], F32, tag="yo",
                                                name=f"yo{ob}_{o}_{t}")
                                nc.vector.tensor_copy(yo[:], pys[o][t][:])
                                nc.sync.dma_start(
                                    yT_d[og:og + 128, t * 512:(t + 1) * 512], yo[:])

    nc.compile()
    return nc


def _get_nc():
    global _CACHED_NC
    if _CACHED_NC is None:
        _CACHED_NC = _build()
    return _CACHED_NC


def _prep_in_maps(x, weight, lora_A, lora_B, token_to_slot):
    x = np.asarray(x, dtype=np.float32)
    weight = np.asarray(weight, dtype=np.float32)
    lora_A = np.asarray(lora_A, dtype=np.float32)
    lora_B = np.asarray(lora_B, dtype=np.float32)
    slots = np.asarray(token_to_slot)

    wT = np.ascontiguousarray(weight.T)                                    # [D_IN, D_OUT]
    aT = np.ascontiguousarray(lora_A.transpose(2, 0, 1).reshape(D_IN, LR))  # [D_IN, L*R]
    bC = np.ascontiguousarray(lora_B.transpose(0, 2, 1).reshape(LR, D_OUT)) # [L*R, D_OUT]

    # One-hot mask over stacked adapter rows; out-of-range slots -> all-zero.
    # bf16 is exact for 0/1 and halves the SBUF footprint.
    maskT = np.zeros((LR, T), dtype=np.float32)
    for l in range(L):
        maskT[l * R:(l + 1) * R, :] = (slots == l).astype(np.float32)[None, :]

    in_maps = []
    for c in range(N_CORES):
        tsl = slice(c * T_SH, (c + 1) * T_SH)
        in_maps.append({
            "xT": np.ascontiguousarray(x[tsl, :].T),
            "wT": wT,
            "aT": aT,
            "bC": bC,
            "maskT": np.ascontiguousarray(maskT[:, tsl]).astype(ml_dtypes.bfloat16),
        })
    return in_maps


def _run(inputs, trace=False, trace_cores=None):
    nc = _get_nc()
    in_maps = _prep_in_maps(**inputs)
    res = run_bass_kernel_spmd(
        nc, in_maps, core_ids=list(range(N_CORES)),
        trace=trace, trace_cores=trace_cores,
    )
    y = np.concatenate([res.results[c]["yT"].T for c in range(N_CORES)], axis=0)
    y = np.ascontiguousarray(y)
    return y, res


def _validate(inputs, y):
    """Cheap host-side sanity check: project y onto a random vector and compare
    with the host-computed projection. Catches the (rare, transient) device
    corruption observed on this setup; costs <1 s on host BLAS."""
    x = np.asarray(inputs["x"], dtype=np.float32)
    weight = np.asarray(inputs["weight"], dtype=np.float32)
    lora_A = np.asarray(inputs["lora_A"], dtype=np.float32)
    lora_B = np.asarray(inputs["lora_B"], dtype=np.float32)
    slots = np.asarray(inputs["token_to_slot"])

    rng = np.random.default_rng(12345)
    r = rng.standard_normal(D_OUT).astype(np.float64)

    base = x.astype(np.float64) @ (weight.astype(np.float64).T @ r)      # [T]
    aT = lora_A.transpose(2, 0, 1).reshape(D_IN, LR)                      # [D_IN, LR]
    bC = lora_B.transpose(0, 2, 1).reshape(LR, D_OUT)                     # [LR, D_OUT]
    u = (x @ aT).astype(np.float64)                                       # [T, LR]
    m = np.zeros((T, LR))
    for l in range(L):
        m[:, l * R:(l + 1) * R] = (slots == l).astype(np.float64)[:, None]
    exp = base + (u * m) @ (bC.astype(np.float64) @ r)                    # [T]
    got = y.astype(np.float64) @ r
    scale = np.abs(exp).max()
    rel = np.abs(got - exp).max() / scale
    return rel < 3e-3


def kernel(x, weight, lora_A, lora_B, token_to_slot):
    inputs = dict(x=x, weight=weight, lora_A=lora_A, lora_B=lora_B,
                  token_to_slot=token_to_slot)
    y = None
    for _attempt in range(3):
        y, _ = _run(inputs)
        if _validate(inputs, y):
            break
    return y


# revision 22
# speedup vs baseline: 1.0439x; 1.0439x over previous
"""LoRA row-parallel linear on 8 TRN2 NeuronCores.

Problem: y = x @ W^T + delta, where per-token LoRA delta[t] = B[s] @ (A[s] @ x[t]),
s = token_to_slot[t] (8 adapters, rank 16, scaling baked into B).

Strategy: token data-parallel across the 8 cores (T=8192 -> 1024 tokens/core).
No collectives needed; each core computes its token block fully, in transposed
output space (y^T, un-transposed on the host):
  u^T   = A_all @ x_shard^T          (128 x T_SH; A_all = all 8 adapters stacked)
  uM^T  = u^T * mask^T               (one-hot select of each token's adapter)
  y^T   = W @ x^T + B_all^T @ uM^T   (PSUM accumulation: 32 k-tiles of W + 1 of B)
All matmuls run as float32r (TF32-like, FP22) at full PE rate with fp32
accumulate (measured ~227 ns per 128x128x512 matmul, the intrinsic pacing;
f32r matmuls are self-loading, so stationary-operand choice is neutral).

Schedule (per core): the first output-column block (ob0) runs its 32-k-tile
d-loop FIRST, so the PE has work while the 16 MB x^T shard streams in; the
u-pass (which needs the whole shard) runs after it, and ob0's LoRA delta is
applied as a separate accumulation + DVE add. Remaining obs fuse the delta as
a 33rd accumulation step.

Host prep: transposes x/W/A to put the contraction dim on partitions, builds
the one-hot mask from token_to_slot. Device does all the FLOPs.
"""

import numpy as np
import ml_dtypes

from concourse import bacc, tile, mybir
from concourse.bass_utils import run_bass_kernel_spmd
import concourse.bass_utils as _bu

# Disable S3 artifact upload in the trace path (no credentials in this container).
_bu.upload_artifacts = lambda tmpdir: "local://" + tmpdir

N_CORES = 8
T = 8192
D_IN = 4096
D_OUT = 4096
L = 8          # max adapters
R = 16         # max rank
LR = L * R     # 128 = stacked adapter dim
T_SH = T // N_CORES          # 1024 tokens per core
KT = D_IN // 128             # 32 contraction tiles
OB = D_OUT // 512            # 8 output-column superblocks
NO = 4                       # 128-wide output blocks per superblock
NT = T_SH // 512             # 2 token blocks (moving dim)

F32 = mybir.dt.float32
F32R = mybir.dt.float32r

_CACHED_NC = None


def _build():
    nc = bacc.Bacc("TRN2", target_bir_lowering=False, debug=False)

    xT_d = nc.dram_tensor("xT", [D_IN, T_SH], F32, kind="ExternalInput")
    wT_d = nc.dram_tensor("wT", [D_IN, D_OUT], F32, kind="ExternalInput")
    aT_d = nc.dram_tensor("aT", [D_IN, LR], F32, kind="ExternalInput")
    bC_d = nc.dram_tensor("bC", [LR, D_OUT], F32, kind="ExternalInput")
    mT_d = nc.dram_tensor("maskT", [LR, T_SH], mybir.dt.bfloat16, kind="ExternalInput")
    yT_d = nc.dram_tensor("yT", [D_OUT, T_SH], F32, kind="ExternalOutput")

    with tile.TileContext(nc) as tc:
        with (
            tc.tile_pool(name="resident", bufs=1) as rpool,
            tc.tile_pool(name="wstream", bufs=9) as wpool,
            tc.tile_pool(name="yout", bufs=3) as ypool,
            tc.tile_pool(name="psum", bufs=8, space="PSUM") as psum,
        ):
            # --- resident loads; xts interleaved with ob0's w tiles so the
            # --- ob0 d-loop can start as soon as the first k-tile lands.
            xts = []
            wts0 = []
            ats = []
            for d in range(KT):
                xt = rpool.tile([128, T_SH], F32R, tag=f"xt{d}")
                nc.sync.dma_start(xt[:], xT_d[d * 128:(d + 1) * 128, :].bitcast(F32R))
                xts.append(xt)
                wt = wpool.tile([128, 512], F32R, tag="wt", name=f"wt0_{d}")
                nc.sync.dma_start(wt[:], wT_d[d * 128:(d + 1) * 128, 0:512].bitcast(F32R))
                wts0.append(wt)
                at = rpool.tile([128, LR], F32R, tag=f"at{d}", name=f"at{d}")
                nc.sync.dma_start(at[:], aT_d[d * 128:(d + 1) * 128, :].bitcast(F32R))
                ats.append(at)
            bc = rpool.tile([LR, D_OUT], F32R, tag="bc")
            nc.sync.dma_start(bc[:], bC_d[:].bitcast(F32R))
            mask = rpool.tile([LR, T_SH], mybir.dt.bfloat16, tag="mask")
            nc.sync.dma_start(mask[:], mT_d[:])
            uTms = [rpool.tile([LR, 512], F32R, tag=f"uTm{ub}", name=f"uTm{ub}")
                    for ub in range(NT)]

            # --- phase 1: ob0 d-loop (base matmul only, no delta) --------------
            # psum tile (o, t) = y^T[o-block of 128, t-block of 512]
            pys0 = [[psum.tile([128, 512], F32, tag="acc", name=f"py0_{o}_{t}")
                     for t in range(NT)] for o in range(NO)]
            yo0s = {}
            for d in range(KT):
                for o in range(NO):
                    lw = wts0[d][:, o * 128:(o + 1) * 128]
                    for t in range(NT):
                        nc.tensor.matmul(
                            pys0[o][t][:], lw, xts[d][:, t * 512:(t + 1) * 512],
                            start=(d == 0), stop=(d == KT - 1), skip_group_check=True,
                        )
                        if d == KT - 1:
                            yo0 = rpool.tile([128, 512], F32, tag=f"yo0_{o}_{t}",
                                             name=f"yo0_{o}_{t}")
                            nc.vector.tensor_copy(yo0[:], pys0[o][t][:])
                            yo0s[o, t] = yo0

            # --- phase 2: u-pass (needs all xts, which have landed by now) -----
            for ub in range(NT):
                pu = psum.tile([128, 512], F32, tag="acc", name=f"pu{ub}")
                sl = slice(ub * 512, (ub + 1) * 512)
                for d in range(KT):
                    nc.tensor.matmul(
                        pu[:], ats[d][:], xts[d][:, sl],
                        start=(d == 0), stop=(d == KT - 1), skip_group_check=True,
                    )
                nc.vector.tensor_mul(uTms[ub][:], pu[:], mask[:, sl])

            # --- phase 3: ob0 delta + writeback --------------------------------
            for o in range(NO):
                for t in range(NT):
                    pd = psum.tile([128, 512], F32, tag="acc", name=f"pd{o}_{t}")
                    nc.tensor.matmul(
                        pd[:], bc[:, o * 128:(o + 1) * 128], uTms[t][:],
                        start=True, stop=True, skip_group_check=True,
                    )
                    yo = ypool.tile([128, 512], F32, tag="yo", name=f"yod{o}_{t}")
                    nc.vector.tensor_add(yo[:], yo0s[o, t][:], pd[:])
                    nc.sync.dma_start(
                        yT_d[o * 128:(o + 1) * 128, t * 512:(t + 1) * 512], yo[:])

            # --- phase 4: ob1..7 with fused delta ------------------------------
            for ob in range(1, OB):
                pys = [[psum.tile([128, 512], F32, tag="acc", name=f"py{ob}_{o}_{t}")
                        for t in range(NT)] for o in range(NO)]
                for d in range(KT):
                    wt = wpool.tile([128, 512], F32R, tag="wt", name=f"wt{ob}_{d}")
                    nc.sync.dma_start(
                        wt[:],
                        wT_d[d * 128:(d + 1) * 128,
                             ob * 512:(ob + 1) * 512].bitcast(F32R))
                    for o in range(NO):
                        lw = wt[:, o * 128:(o + 1) * 128]
                        og = ob * 512 + o * 128
                        for t in range(NT):
                            nc.tensor.matmul(
                                pys[o][t][:], lw, xts[d][:, t * 512:(t + 1) * 512],
                                start=(d == 0), stop=False, skip_group_check=True,
                            )
                            if d == KT - 1:
                                nc.tensor.matmul(
                                    pys[o][t][:], bc[:, og:og + 128], uTms[t][:],
                                    start=False, stop=True, skip_group_check=True,
                                )
                                yo = ypool.tile([128, 512], F32, tag="yo",
                                                name=f"yo{ob}_{o}_{t}")
                                nc.vector.tensor_copy(yo[:], pys[o][t][:])
                                nc.sync.dma_start(
                                    yT_d[og:og + 128, t * 512:(t + 1) * 512], yo[:])

    nc.compile()
    return nc


def _get_nc():
    global _CACHED_NC
    if _CACHED_NC is None:
        _CACHED_NC = _build()
    return _CACHED_NC


def _prep_in_maps(x, weight, lora_A, lora_B, token_to_slot):
    x = np.asarray(x, dtype=np.float32)
    weight = np.asarray(weight, dtype=np.float32)
    lora_A = np.asarray(lora_A, dtype=np.float32)
    lora_B = np.asarray(lora_B, dtype=np.float32)
    slots = np.asarray(token_to_slot)

    wT = np.ascontiguousarray(weight.T)                                    # [D_IN, D_OUT]
    aT = np.ascontiguousarray(lora_A.transpose(2, 0, 1).reshape(D_IN, LR))  # [D_IN, L*R]
    bC = np.ascontiguousarray(lora_B.transpose(0, 2, 1).reshape(LR, D_OUT)) # [L*R, D_OUT]

    # One-hot mask over stacked adapter rows; out-of-range slots -> all-zero.
    # bf16 is exact for 0/1 and halves the SBUF footprint.
    maskT = np.zeros((LR, T), dtype=np.float32)
    for l in range(L):
        maskT[l * R:(l + 1) * R, :] = (slots == l).astype(np.float32)[None, :]

    in_maps = []
    for c in range(N_CORES):
        tsl = slice(c * T_SH, (c + 1) * T_SH)
        in_maps.append({
            "xT": np.ascontiguousarray(x[tsl, :].T),
            "wT": wT,
            "aT": aT,
            "bC": bC,
            "maskT": np.ascontiguousarray(maskT[:, tsl]).astype(ml_dtypes.bfloat16),
        })
    return in_maps


def _run(inputs, trace=False, trace_cores=None):
    nc = _get_nc()
    in_maps = _prep_in_maps(**inputs)
    res = run_bass_kernel_spmd(
        nc, in_maps, core_ids=list(range(N_CORES)),
        trace=trace, trace_cores=trace_cores,
    )
    y = np.concatenate([res.results[c]["yT"].T for c in range(N_CORES)], axis=0)
    y = np.ascontiguousarray(y)
    return y, res


def _validate(inputs, y):
    """Cheap host-side sanity check: project y onto a random vector and compare
    with the host-computed projection. Catches the (rare, transient) device
    corruption observed on this setup; costs <1 s on host BLAS."""
    x = np.asarray(inputs["x"], dtype=np.float32)
    weight = np.asarray(inputs["weight"], dtype=np.float32)
    lora_A = np.asarray(inputs["lora_A"], dtype=np.float32)
    lora_B = np.asarray(inputs["lora_B"], dtype=np.float32)
    slots = np.asarray(inputs["token_to_slot"])

    rng = np.random.default_rng(12345)
    r = rng.standard_normal(D_OUT).astype(np.float64)

    base = x.astype(np.float64) @ (weight.astype(np.float64).T @ r)      # [T]
    aT = lora_A.transpose(2, 0, 1).reshape(D_IN, LR)                      # [D_IN, LR]
    bC = lora_B.transpose(0, 2, 1).reshape(LR, D_OUT)                     # [LR, D_OUT]
    u = (x @ aT).astype(np.float64)                                       # [T, LR]
    m = np.zeros((T, LR))
    for l in range(L):
        m[:, l * R:(l + 1) * R] = (slots == l).astype(np.float64)[:, None]
    exp = base + (u * m) @ (bC.astype(np.float64) @ r)                    # [T]
    got = y.astype(np.float64) @ r
    scale = np.abs(exp).max()
    rel = np.abs(got - exp).max() / scale
    return rel < 3e-3


def kernel(x, weight, lora_A, lora_B, token_to_slot):
    inputs = dict(x=x, weight=weight, lora_A=lora_A, lora_B=lora_B,
                  token_to_slot=token_to_slot)
    y = None
    for _attempt in range(3):
        y, _ = _run(inputs)
        if _validate(inputs, y):
            break
    return y
